# revision 11
# baseline (speedup 1.0000x reference)
"""Trainium2 Bass kernel for nn_PhyloDisentangler (8-core SPMD).

Sharding: tensor-parallel on the two big MLP weights (16384x4096), each core
owning 4 of the 32 VQ code slots (mlp_in rows, ordered (sl, ed)) and 2048 of
the 16384 mlp_out rows.  Data-parallel (8 batches/core) conv_in+LN and
conv_out.  Collectives: AllGather of LN output (k-major), AllGather of
quantized codes, AllReduce of the VQ loss scalar, AllToAll of the mlp_out
activations back to batch sharding.
"""
import numpy as np

import concourse.bass as bass
import concourse.mybir as mybir
import concourse.tile as tile
from concourse import bacc
from concourse.bass_utils import run_bass_kernel_spmd
from concourse.masks import make_identity

B = 64; C_IN = 256; CH = 128; NPH = 64; R = 16; ED = 128; CB = 8; NL = 4
NE = 1024; C_OUT = 256; BETA = 0.25; LN_EPS = 1e-5
PX = R * R                      # 256
FLAT_PH = NPH * PX              # 16384
FLAT_Z = ED * CB * NL           # 4096
NCORES = 8
BPC = B // NCORES               # 8  batches per core
SPC = (CB * NL) // NCORES       # 4  code slots per core
ZPC = ED * SPC                  # 512 z dims per core
HPC = FLAT_PH // NCORES         # 2048 mlp_out rows per core
NPIX = BPC * PX                 # 2048 pixels per core
KT_IN = FLAT_PH // 128          # 128 k-tiles for mlp_in
KT_OUT = FLAT_Z // 128          # 32 k-tiles for mlp_out

f32 = mybir.dt.float32
u32 = mybir.dt.uint32
AX = mybir.AxisListType
AF = mybir.ActivationFunctionType


def _body(tc, t, ctx):
    nc = tc.nc
    ident_pool = ctx.enter_context(tc.tile_pool(name="const", bufs=1))
    sb = ctx.enter_context(tc.tile_pool(name="sb", bufs=1))
    wpool = ctx.enter_context(tc.tile_pool(name="wstream", bufs=4))
    lpool = ctx.enter_context(tc.tile_pool(name="lstream", bufs=4))
    ps = ctx.enter_context(tc.tile_pool(name="ps", bufs=2, space="PSUM"))
    ps_acc = ctx.enter_context(tc.tile_pool(name="ps_acc", bufs=4, space="PSUM"))

    ident = ident_pool.tile([128, 128], f32)
    make_identity(nc, ident[:])

    # ---------------- Stage A: conv_in (DP over batch) ----------------
    x_sb = sb.tile([128, 2, NPIX], f32)
    nc.sync.dma_start(out=x_sb[:], in_=t["x_t"].rearrange("(ko ki) n -> ki ko n", ki=128))
    sx_sb = sb.tile([128, 2, NPIX], f32)
    nc.scalar.activation(sx_sb[:], x_sb[:], AF.Silu)
    ciw_sb = sb.tile([128, 2, CH], f32)
    nc.sync.dma_start(out=ciw_sb[:], in_=t["conv_in_wT"].rearrange("(ko ki) m -> ki ko m", ki=128))
    cib_sb = sb.tile([128, 1], f32)
    nc.sync.dma_start(out=cib_sb[:], in_=t["conv_in_b"][:, None])

    h_ph = sb.tile([NPH, BPC, PX], f32)      # phylo channels, pre-LN
    hcat = sb.tile([128, BPC, PX], f32)      # conv_out input (img half filled now)
    for nb in range(4):
        pch = ps.tile([128, 512], f32, name="pconv", tag="pgen")
        for ko in range(2):
            nc.tensor.matmul(pch[:], ciw_sb[:, ko, :], sx_sb[:, ko, bass.ts(nb, 512)],
                             start=(ko == 0), stop=(ko == 1))
        hv = h_ph.rearrange("c b p -> c (b p)")
        cv = hcat.rearrange("c b p -> c (b p)")
        nc.vector.tensor_add(out=hv[:, bass.ts(nb, 512)], in0=pch[:NPH],
                             in1=cib_sb[:NPH].to_broadcast([NPH, 512]))
        nc.vector.tensor_add(out=cv[NPH:, bass.ts(nb, 512)], in0=pch[NPH:],
                             in1=cib_sb[NPH:].to_broadcast([NPH, 512]))

    # ---------------- Stage B: LayerNorm over [NPH,R,R] per batch ----------------
    ones64 = sb.tile([NPH, 1], f32)
    nc.vector.memset(ones64[:], 1.0)
    h2 = sb.tile([NPH, BPC, PX], f32)
    nc.vector.tensor_mul(out=h2[:], in0=h_ph[:], in1=h_ph[:])
    hs2 = sb.tile([NPH, 2, BPC], f32)
    nc.vector.reduce_sum(hs2[:, 0, :], h_ph[:], axis=AX.X)
    nc.vector.reduce_sum(hs2[:, 1, :], h2[:], axis=AX.X)
    pst = ps.tile([1, 2 * BPC], f32, name="pst", tag="pgen")
    nc.tensor.matmul(pst[:], ones64[:], hs2.rearrange("c t b -> c (t b)")[:],
                     start=True, stop=True)
    st8 = sb.tile([1, 2, BPC], f32)
    nc.vector.tensor_copy(out=st8.rearrange("o t b -> o (t b)")[:], in_=pst[:])
    # mu = s/N ; msq = ss/N ; var = msq - mu^2 ; a = 1/sqrt(var+eps) ; bneg = -mu*a
    stats = sb.tile([1, 2, BPC], f32)        # [0]=a, [1]=bneg
    mu = sb.tile([1, BPC], f32)
    msq = sb.tile([1, BPC], f32)
    nc.vector.tensor_scalar_mul(mu[:], st8[:, 0, :], 1.0 / FLAT_PH)
    nc.vector.tensor_scalar_mul(msq[:], st8[:, 1, :], 1.0 / FLAT_PH)
    var = sb.tile([1, BPC], f32)
    nc.vector.tensor_mul(out=var[:], in0=mu[:], in1=mu[:])
    nc.vector.tensor_sub(out=var[:], in0=msq[:], in1=var[:])
    sq = sb.tile([1, BPC], f32)
    nc.vector.tensor_scalar_add(var[:], var[:], LN_EPS)
    nc.scalar.activation(sq[:], var[:], AF.Sqrt)
    nc.vector.reciprocal(stats[:, 0, :], sq[:])
    nc.vector.tensor_mul(out=stats[:, 1, :], in0=mu[:], in1=stats[:, 0, :])
    nc.vector.tensor_scalar_mul(stats[:, 1, :], stats[:, 1, :], -1.0)
    # broadcast stats to all 64 partitions via ones-matmul
    ones1 = sb.tile([1, NPH], f32)
    nc.vector.memset(ones1[:], 1.0)
    psum_bc = ps.tile([NPH, 2 * BPC], f32, name="psum_bc", tag="pgen")
    nc.tensor.matmul(psum_bc[:], ones1[:], stats.rearrange("o t b -> o (t b)")[:],
                     start=True, stop=True)
    bc = sb.tile([NPH, 2, BPC], f32)
    nc.vector.tensor_copy(out=bc.rearrange("c t b -> c (t b)")[:], in_=psum_bc[:])
    # hn = (h*a + bneg) * ln_w + ln_b
    lnw_sb = sb.tile([NPH, PX], f32)
    nc.sync.dma_start(out=lnw_sb[:], in_=t["ln_w"].rearrange("(c p) -> c p", c=NPH))
    lnb_sb = sb.tile([NPH, PX], f32)
    nc.sync.dma_start(out=lnb_sb[:], in_=t["ln_b"].rearrange("(c p) -> c p", c=NPH))
    hn = sb.tile([NPH, BPC, PX], f32)
    nc.vector.tensor_mul(out=hn[:], in0=h_ph[:], in1=bc[:, 0, :, None].to_broadcast([NPH, BPC, PX]))
    nc.vector.tensor_add(out=hn[:], in0=hn[:], in1=bc[:, 1, :, None].to_broadcast([NPH, BPC, PX]))
    nc.vector.tensor_mul(out=hn[:], in0=hn[:], in1=lnw_sb[:, None, :].to_broadcast([NPH, BPC, PX]))
    nc.vector.tensor_add(out=hn[:], in0=hn[:], in1=lnb_sb[:, None, :].to_broadcast([NPH, BPC, PX]))

    # ---------------- Stage C: transpose hn -> k-major shard; AllGather ----------------
    hnT = sb.tile([128, NPH, 2, BPC], f32)   # [px_local, ch, half, b]
    for b in range(BPC):
        for hh in range(2):
            pt = ps.tile([128, NPH], f32, name="pt_hn", tag="pgen")
            nc.tensor.transpose(pt[:], hn[:, b, bass.ts(hh, 128)], ident[:NPH, :NPH])
            nc.vector.tensor_copy(out=hnT[:, :, hh, b], in_=pt[:])
    nc.sync.dma_start(
        out=t["hn_shard"].rearrange("(c hh ki) b -> ki c hh b", c=NPH, hh=2, ki=128),
        in_=hnT[:])
    rg = [list(range(NCORES))]
    nc.gpsimd.collective_compute(
        "AllGather", mybir.AluOpType.bypass, replica_groups=rg,
        ins=[t["hn_shard"][:]], outs=[t["g_hn"][:]])

    # ---------------- Stage D: mlp_in (TP): z[64, 512] ----------------
    g_hn_v = t["g_hn"].rearrange("(c ko ki) b -> ko ki c b", c=NCORES, ko=KT_IN, ki=128)
    w_in_v = t["w_inT"].rearrange("(ko ki) n -> ko ki n", ki=128)
    psum_z = ps_acc.tile([B, ZPC], f32, name="psum_z", tag="acc")
    for ko in range(KT_IN):
        lh = lpool.tile([128, NCORES, BPC], f32, name="lh_in", tag="lh_in")
        nc.sync.dma_start(out=lh[:], in_=g_hn_v[ko])
        rh = wpool.tile([128, ZPC], f32, name="rh_in", tag="rh_in")
        nc.sync.dma_start(out=rh[:], in_=w_in_v[ko])
        nc.tensor.matmul(psum_z[:], lh.rearrange("k c b -> k (c b)")[:], rh[:],
                         start=(ko == 0), stop=(ko == KT_IN - 1))
    z_sb = sb.tile([B, ZPC], f32)
    bin_sb = sb.tile([B, ZPC], f32)
    nc.sync.dma_start(out=bin_sb[:], in_=t["b_in"][:])
    nc.vector.tensor_add(out=z_sb[:], in0=psum_z[:], in1=bin_sb[:])

    # ---------------- Stage E: VQ on local slots ----------------
    zT = sb.tile([ED, SPC, B], f32)
    for sl in range(SPC):
        pt = ps.tile([ED, B], f32, name="pt_z", tag="pgen")
        nc.tensor.transpose(pt[:], z_sb[:, bass.ts(sl, ED)], ident[:B, :B])
        nc.vector.tensor_copy(out=zT[:, sl, :], in_=pt[:])
    cbT_sb = sb.tile([ED, NE], f32)
    nc.sync.dma_start(out=cbT_sb[:], in_=t["cbT"][:])
    cn_sb = sb.tile([B, NE], f32)
    nc.sync.dma_start(out=cn_sb[:], in_=t["cnorm"][:])
    zqT = sb.tile([ED, SPC, B], f32)
    for sl in range(SPC):
        psc = ps.tile([B, NE], f32, name="psc", tag="psc", bufs=1)
        for nb in range(2):
            nc.tensor.matmul(psc[:, bass.ts(nb, 512)], zT[:, sl, :], cbT_sb[:, bass.ts(nb, 512)],
                             start=True, stop=True)
        scores = sb.tile([B, NE], f32, name="scores", tag="scores")
        nc.vector.tensor_sub(out=scores[:], in0=psc[:], in1=cn_sb[:])
        mx8 = sb.tile([B, 8], f32, name="mx8", tag="mx8")
        nc.vector.max(out=mx8[:], in_=scores[:])
        idx8 = sb.tile([B, 8], u32, name="idx8", tag="idx8")
        nc.vector.max_index(idx8[:], mx8[:], scores[:])
        zq_sl = sb.tile([B, ED], f32, name="zq_sl", tag="zq_sl")
        nc.gpsimd.indirect_dma_start(
            out=zq_sl[:], out_offset=None, in_=t["cb"][:],
            in_offset=bass.IndirectOffsetOnAxis(ap=idx8[:, :1], axis=0))
        pt2 = ps.tile([ED, B], f32, name="pt_zq", tag="pgen")
        nc.tensor.transpose(pt2[:], zq_sl[:], ident[:B, :B])
        nc.vector.tensor_copy(out=zqT[:, sl, :], in_=pt2[:])
    # loss partial: sum((zq - z)^2) * (1+beta)/numel  -> AllReduce
    dif = sb.tile([ED, SPC, B], f32)
    nc.vector.tensor_sub(out=dif[:], in0=zqT[:], in1=zT[:])
    nc.vector.tensor_mul(out=dif[:], in0=dif[:], in1=dif[:])
    dred = sb.tile([ED, 1], f32)
    nc.vector.reduce_sum(dred[:], dif.rearrange("e s b -> e (s b)")[:], axis=AX.X)
    ones128 = sb.tile([128, 1], f32)
    nc.vector.memset(ones128[:], 1.0)
    pl = ps.tile([1, 1], f32, name="pl", tag="pgen")
    nc.tensor.matmul(pl[:], ones128[:], dred[:], start=True, stop=True)
    lp_sb = sb.tile([1, 1], f32)
    nc.vector.tensor_scalar_mul(lp_sb[:], pl[:], (1.0 + BETA) / (B * FLAT_Z))
    nc.sync.dma_start(out=t["loss_part"][:], in_=lp_sb[:])
    # AG zq (also k-major): shard row r = sl*128 + ed -> global k = c*512 + r
    nc.sync.dma_start(out=t["zq_shard"].rearrange("(sl e) b -> e sl b", e=ED), in_=zqT[:])
    nc.gpsimd.collective_compute(
        "AllGather", mybir.AluOpType.bypass, replica_groups=rg,
        ins=[t["zq_shard"][:]], outs=[t["g_zq"][:]])
    nc.gpsimd.collective_compute(
        "AllReduce", mybir.AluOpType.add, replica_groups=rg,
        ins=[t["loss_part"][:]], outs=[t["loss_red"][:]])
    loss_sb = sb.tile([1, 1], f32)
    nc.sync.dma_start(out=loss_sb[:], in_=t["loss_red"][:])
    nc.sync.dma_start(out=t["loss_out"][:], in_=loss_sb[:])

    # ---------------- Stage F: mlp_out (TP): hout[64, 2048] ----------------
    g_zq_v = t["g_zq"].rearrange("(ko ki) b -> ko ki b", ki=128)
    w_out_v = t["w_outT"].rearrange("(ko ki) n -> ko ki n", ki=128)
    psum_h = [ps_acc.tile([B, 512], f32, name=f"psum_h{i}", tag="acc")
              for i in range(4)]
    for ko in range(KT_OUT):
        lh = lpool.tile([128, B], f32, name="lh_out", tag="lh_out")
        nc.sync.dma_start(out=lh[:], in_=g_zq_v[ko])
        for nb in range(4):
            rh = wpool.tile([128, 512], f32, name="rh_out", tag="rh_out")
            nc.sync.dma_start(out=rh[:], in_=w_out_v[ko, :, bass.ts(nb, 512)])
            nc.tensor.matmul(psum_h[nb][:], lh[:], rh[:],
                             start=(ko == 0), stop=(ko == KT_OUT - 1))
    hout = sb.tile([B, HPC], f32)
    bout_sb = sb.tile([B, HPC], f32)
    nc.sync.dma_start(out=bout_sb[:], in_=t["b_out"][:])
    for nb in range(4):
        nc.vector.tensor_add(out=hout[:, bass.ts(nb, 512)], in0=psum_h[nb][:],
                             in1=bout_sb[:, bass.ts(nb, 512)])
    nc.sync.dma_start(out=t["a2a_in"].rearrange("j bl r -> (j bl) r"), in_=hout[:])
    nc.gpsimd.collective_compute(
        "AllToAll", mybir.AluOpType.bypass, replica_groups=rg,
        ins=[t["a2a_in"][:]], outs=[t["a2a_out"][:]])

    # ---------------- Stage G: conv_out (DP over batch) ----------------
    for j in range(NCORES):
        nc.sync.dma_start(
            out=hcat[bass.ts(j, 8), :, :],
            in_=t["a2a_out"][j].rearrange("bl (ch p) -> ch bl p", ch=8))
    scat = sb.tile([128, BPC, PX], f32)
    nc.scalar.activation(scat[:], hcat[:], AF.Silu)
    cow_sb = sb.tile([128, C_OUT], f32)
    nc.sync.dma_start(out=cow_sb[:], in_=t["conv_out_wT"][:])
    cob_sb = sb.tile([128, 2], f32)
    nc.sync.dma_start(out=cob_sb[:], in_=t["conv_out_b"].rearrange("(mt p) -> p mt", p=128))
    sv = scat.rearrange("c b p -> c (b p)")
    for mt in range(2):
        out_mt = sb.tile([128, BPC, PX], f32, name=f"out_mt{mt}", tag=f"out_mt{mt}")
        ov = out_mt.rearrange("o b p -> o (b p)")
        for nb in range(4):
            po = ps.tile([128, 512], f32, name="po", tag="pgen")
            nc.tensor.matmul(po[:], cow_sb[:, bass.ts(mt, 128)], sv[:, bass.ts(nb, 512)],
                             start=True, stop=True)
            nc.vector.tensor_add(out=ov[:, bass.ts(nb, 512)], in0=po[:],
                                 in1=cob_sb[:, mt:mt + 1].to_broadcast([128, 512]))
        nc.sync.dma_start(
            out=t["out_c"][:, bass.ts(mt, 128), :].rearrange("b o p -> o b p"),
            in_=out_mt[:])


def build_program():
    nc = bacc.Bacc("TRN2", target_bir_lowering=False, debug=False,
                   num_devices=NCORES)
    t = {}
    def inp(name, shape):
        t[name] = nc.dram_tensor(name, shape, f32, kind="ExternalInput").ap()
    inp("x_t", [C_IN, NPIX])
    inp("conv_in_wT", [C_IN, CH])
    inp("conv_in_b", [CH])
    inp("ln_w", [FLAT_PH])
    inp("ln_b", [FLAT_PH])
    inp("w_inT", [FLAT_PH, ZPC])
    inp("b_in", [B, ZPC])
    inp("cbT", [ED, NE])
    inp("cb", [NE, ED])
    inp("cnorm", [B, NE])
    inp("w_outT", [FLAT_Z, HPC])
    inp("b_out", [B, HPC])
    inp("conv_out_wT", [CH, C_OUT])
    inp("conv_out_b", [C_OUT])
    t["out_c"] = nc.dram_tensor("out_c", [BPC, C_OUT, PX], f32, kind="ExternalOutput").ap()
    t["loss_out"] = nc.dram_tensor("loss_out", [1, 1], f32, kind="ExternalOutput").ap()
    # internal / shared DRAM for collectives
    t["hn_shard"] = nc.dram_tensor("hn_shard", [FLAT_PH, BPC], f32).ap()
    t["g_hn"] = nc.dram_tensor("g_hn", [NCORES * FLAT_PH, BPC], f32, addr_space="Shared").ap()
    t["zq_shard"] = nc.dram_tensor("zq_shard", [ZPC, B], f32).ap()
    t["g_zq"] = nc.dram_tensor("g_zq", [FLAT_Z, B], f32, addr_space="Shared").ap()
    t["loss_part"] = nc.dram_tensor("loss_part", [1, 1], f32).ap()
    t["loss_red"] = nc.dram_tensor("loss_red", [1, 1], f32, addr_space="Shared").ap()
    t["a2a_in"] = nc.dram_tensor("a2a_in", [NCORES, BPC, HPC], f32).ap()
    t["a2a_out"] = nc.dram_tensor("a2a_out", [NCORES, BPC, HPC], f32).ap()

    from contextlib import ExitStack
    with tile.TileContext(nc) as tc:
        with ExitStack() as ctx:
            _body(tc, t, ctx)
    nc.compile()
    return nc


def host_prep(inputs):
    """Build the 8 per-core input maps from the full-model inputs."""
    f = np.float32
    def c(a):
        return np.ascontiguousarray(a, dtype=f)
    common = {
        "conv_in_wT": c(inputs["conv_in_w"].T),
        "conv_in_b": c(inputs["conv_in_b"]),
        "ln_w": c(inputs["ln_w"].reshape(FLAT_PH)),
        "ln_b": c(inputs["ln_b"].reshape(FLAT_PH)),
        "cbT": c(inputs["codebook"].T),
        "cb": c(inputs["codebook"]),
        "cnorm": c(np.broadcast_to(
            0.5 * (inputs["codebook"].astype(np.float64) ** 2).sum(1).astype(f)[None, :],
            (B, NE))),
        "conv_out_wT": c(inputs["conv_out_w"].T),
        "conv_out_b": c(inputs["conv_out_b"]),
    }
    w_in = inputs["mlp_in_w"]
    w_out = inputs["mlp_out_w"]
    col_order = np.array([ed * 32 + 4 * cc + sl
                          for cc in range(NCORES) for sl in range(SPC) for ed in range(ED)])
    w_out_perm = w_out[:, col_order]
    maps = []
    for cc in range(NCORES):
        d = dict(common)
        xs = inputs["x"][cc * BPC:(cc + 1) * BPC].reshape(BPC, C_IN, PX)
        d["x_t"] = c(xs.transpose(1, 0, 2).reshape(C_IN, NPIX))
        rows = np.array([ed * 32 + 4 * cc + sl for sl in range(SPC) for ed in range(ED)])
        d["w_inT"] = c(w_in[rows, :].T)
        d["b_in"] = c(np.broadcast_to(inputs["mlp_in_b"][rows][None, :], (B, ZPC)))
        d["w_outT"] = c(w_out_perm[cc * HPC:(cc + 1) * HPC, :].T)
        d["b_out"] = c(np.broadcast_to(inputs["mlp_out_b"][cc * HPC:(cc + 1) * HPC][None, :], (B, HPC)))
        maps.append(d)
    return maps


_STATE = {}


def kernel(**inputs):
    if "nc" not in _STATE:
        _STATE["nc"] = build_program()
    nc = _STATE["nc"]
    maps = host_prep(inputs)
    res = run_bass_kernel_spmd(nc, maps, core_ids=list(range(NCORES)))
    _STATE["last_res"] = res
    out = np.empty((B, C_OUT, R, R), dtype=np.float32)
    for cc in range(NCORES):
        out[cc * BPC:(cc + 1) * BPC] = res.results[cc]["out_c"].reshape(BPC, C_OUT, R, R)
    loss = np.float32(res.results[0]["loss_out"][0, 0])
    return out, loss


# revision 17
# speedup vs baseline: 1.5479x; 1.5479x over previous
"""Trainium2 Bass kernel for nn_PhyloDisentangler (8-core SPMD).

Sharding: tensor-parallel on the two big MLP weights (16384x4096), each core
owning 4 of the 32 VQ code slots (mlp_in rows, ordered (sl, ed)) and 2048 of
the 16384 mlp_out rows.  Data-parallel (8 batches/core) conv_in+LN and
conv_out.  Collectives: AllGather of LN output (k-major), AllGather of
quantized codes, AllReduce of the VQ loss scalar, AllToAll of the mlp_out
activations back to batch sharding.
"""
import numpy as np

import concourse.bass as bass
import concourse.mybir as mybir
import concourse.tile as tile
from concourse import bacc
from concourse.bass_utils import run_bass_kernel_spmd
from concourse.masks import make_identity

B = 64; C_IN = 256; CH = 128; NPH = 64; R = 16; ED = 128; CB = 8; NL = 4
NE = 1024; C_OUT = 256; BETA = 0.25; LN_EPS = 1e-5
PX = R * R                      # 256
FLAT_PH = NPH * PX              # 16384
FLAT_Z = ED * CB * NL           # 4096
NCORES = 8
BPC = B // NCORES               # 8  batches per core
SPC = (CB * NL) // NCORES       # 4  code slots per core
ZPC = ED * SPC                  # 512 z dims per core
HPC = FLAT_PH // NCORES         # 2048 mlp_out rows per core
NPIX = BPC * PX                 # 2048 pixels per core
KT_IN = FLAT_PH // 128          # 128 k-tiles for mlp_in
KT_OUT = FLAT_Z // 128          # 32 k-tiles for mlp_out

f32 = mybir.dt.float32
f32r = mybir.dt.float32r
u32 = mybir.dt.uint32
AX = mybir.AxisListType
AF = mybir.ActivationFunctionType


def _body(tc, t, ctx):
    nc = tc.nc
    ident_pool = ctx.enter_context(tc.tile_pool(name="const", bufs=1))
    sb = ctx.enter_context(tc.tile_pool(name="sb", bufs=1))
    wpool = ctx.enter_context(tc.tile_pool(name="wstream", bufs=8))
    ps = ctx.enter_context(tc.tile_pool(name="ps", bufs=2, space="PSUM"))
    ps_acc = ctx.enter_context(tc.tile_pool(name="ps_acc", bufs=4, space="PSUM"))

    ident = ident_pool.tile([128, 128], f32)
    make_identity(nc, ident[:])

    # ---------------- Stage A: conv_in (DP over batch) ----------------
    sx_sb = sb.tile([128, 2, NPIX], f32)
    nc.sync.dma_start(out=sx_sb[:], in_=t["x_t"].rearrange("(ko ki) n -> ki ko n", ki=128))
    nc.scalar.activation(sx_sb[:], sx_sb[:], AF.Silu)
    ciw_sb = sb.tile([128, 2, CH], f32)
    nc.sync.dma_start(out=ciw_sb[:], in_=t["conv_in_wT"].rearrange("(ko ki) m -> ki ko m", ki=128))
    cib_sb = sb.tile([128, 1], f32)
    nc.sync.dma_start(out=cib_sb[:], in_=t["conv_in_b"][:, None])

    h_ph = sb.tile([NPH, BPC, PX], f32)      # phylo channels, pre-LN
    hcat = sb.tile([128, BPC, PX], f32)      # conv_out input (img half filled now)
    for nb in range(4):
        pch = ps.tile([128, 512], f32, name="pconv", tag="pgen")
        for ko in range(2):
            nc.tensor.matmul(pch[:], ciw_sb[:, ko, :], sx_sb[:, ko, bass.ts(nb, 512)],
                             start=(ko == 0), stop=(ko == 1))
        hv = h_ph.rearrange("c b p -> c (b p)")
        cv = hcat.rearrange("c b p -> c (b p)")
        nc.vector.tensor_add(out=hv[:, bass.ts(nb, 512)], in0=pch[:NPH],
                             in1=cib_sb[:NPH].to_broadcast([NPH, 512]))
        nc.vector.tensor_add(out=cv[NPH:, bass.ts(nb, 512)], in0=pch[NPH:],
                             in1=cib_sb[NPH:].to_broadcast([NPH, 512]))

    # ---------------- Stage B: LayerNorm over [NPH,R,R] per batch ----------------
    ones64 = sb.tile([NPH, 1], f32)
    nc.vector.memset(ones64[:], 1.0)
    h2 = sb.tile([NPH, BPC, PX], f32, name="h2", tag="h2hn")
    nc.vector.tensor_mul(out=h2[:], in0=h_ph[:], in1=h_ph[:])
    hs2 = sb.tile([NPH, 2, BPC], f32)
    nc.vector.reduce_sum(hs2[:, 0, :], h_ph[:], axis=AX.X)
    nc.vector.reduce_sum(hs2[:, 1, :], h2[:], axis=AX.X)
    pst = ps.tile([1, 2 * BPC], f32, name="pst", tag="pgen")
    nc.tensor.matmul(pst[:], ones64[:], hs2.rearrange("c t b -> c (t b)")[:],
                     start=True, stop=True)
    st8 = sb.tile([1, 2, BPC], f32)
    nc.vector.tensor_copy(out=st8.rearrange("o t b -> o (t b)")[:], in_=pst[:])
    # mu = s/N ; msq = ss/N ; var = msq - mu^2 ; a = 1/sqrt(var+eps) ; bneg = -mu*a
    stats = sb.tile([1, 2, BPC], f32)        # [0]=a, [1]=bneg
    mu = sb.tile([1, BPC], f32)
    msq = sb.tile([1, BPC], f32)
    nc.vector.tensor_scalar_mul(mu[:], st8[:, 0, :], 1.0 / FLAT_PH)
    nc.vector.tensor_scalar_mul(msq[:], st8[:, 1, :], 1.0 / FLAT_PH)
    var = sb.tile([1, BPC], f32)
    nc.vector.tensor_mul(out=var[:], in0=mu[:], in1=mu[:])
    nc.vector.tensor_sub(out=var[:], in0=msq[:], in1=var[:])
    sq = sb.tile([1, BPC], f32)
    nc.vector.tensor_scalar_add(var[:], var[:], LN_EPS)
    nc.scalar.activation(sq[:], var[:], AF.Sqrt)
    nc.vector.reciprocal(stats[:, 0, :], sq[:])
    nc.vector.tensor_mul(out=stats[:, 1, :], in0=mu[:], in1=stats[:, 0, :])
    nc.vector.tensor_scalar_mul(stats[:, 1, :], stats[:, 1, :], -1.0)
    # broadcast stats to all 64 partitions via ones-matmul
    ones1 = sb.tile([1, NPH], f32)
    nc.vector.memset(ones1[:], 1.0)
    psum_bc = ps.tile([NPH, 2 * BPC], f32, name="psum_bc", tag="pgen")
    nc.tensor.matmul(psum_bc[:], ones1[:], stats.rearrange("o t b -> o (t b)")[:],
                     start=True, stop=True)
    bc = sb.tile([NPH, 2, BPC], f32)
    nc.vector.tensor_copy(out=bc.rearrange("c t b -> c (t b)")[:], in_=psum_bc[:])
    # hn = (h*a + bneg) * ln_w + ln_b
    lnw_sb = sb.tile([NPH, PX], f32)
    nc.sync.dma_start(out=lnw_sb[:], in_=t["ln_w"].rearrange("(c p) -> c p", c=NPH))
    lnb_sb = sb.tile([NPH, PX], f32)
    nc.sync.dma_start(out=lnb_sb[:], in_=t["ln_b"].rearrange("(c p) -> c p", c=NPH))
    hn = sb.tile([NPH, BPC, PX], f32, name="hn", tag="h2hn")
    nc.vector.tensor_mul(out=hn[:], in0=h_ph[:], in1=bc[:, 0, :, None].to_broadcast([NPH, BPC, PX]))
    nc.vector.tensor_add(out=hn[:], in0=hn[:], in1=bc[:, 1, :, None].to_broadcast([NPH, BPC, PX]))
    nc.vector.tensor_mul(out=hn[:], in0=hn[:], in1=lnw_sb[:, None, :].to_broadcast([NPH, BPC, PX]))
    nc.vector.tensor_add(out=hn[:], in0=hn[:], in1=lnb_sb[:, None, :].to_broadcast([NPH, BPC, PX]))

    # ---------------- Stage C: transpose hn -> k-major shard; AllGather ----------------
    hnT = sb.tile([128, NPH, 2, BPC], f32)   # [px_local, ch, half, b]
    for b in range(BPC):
        for hh in range(2):
            pt = ps.tile([128, NPH], f32, name="pt_hn", tag="pgen")
            nc.tensor.transpose(pt[:], hn[:, b, bass.ts(hh, 128)], ident[:NPH, :NPH])
            nc.vector.tensor_copy(out=hnT[:, :, hh, b], in_=pt[:])
    nc.sync.dma_start(
        out=t["hn_shard"].rearrange("(ki ko) b -> ki ko b", ki=128),
        in_=hnT.rearrange("p ch hh b -> p (ch hh) b")[:])
    rg = [list(range(NCORES))]
    nc.gpsimd.collective_compute(
        "AllGather", mybir.AluOpType.bypass, replica_groups=rg,
        ins=[t["hn_shard"][:]], outs=[t["g_hn"][:]])

    # ---------------- Stage D: mlp_in (TP): z[64, 512] ----------------
    # G_hn rows are (c, ki, ko): per-partition contiguous 4KB runs per c
    g_hn_v = t["g_hn"].rearrange("(c ki ko) b -> ki c ko b", c=NCORES, ki=128, ko=KT_IN)
    hnT_full = sb.tile([128, NCORES, KT_IN, BPC], f32r)
    for cc in range(NCORES):
        nc.sync.dma_start(out=hnT_full[:, cc], in_=g_hn_v[:, cc].bitcast(f32r))
    w_in_v = t["w_inT"].rearrange("(ko ki) n -> ko ki n", ki=128)
    psum_zT = [ps_acc.tile([ED, B], f32, name=f"psum_zT{i}", tag="acc")
               for i in range(SPC)]
    for ko in range(KT_IN):
        rh = wpool.tile([128, ZPC], f32r, name="rh_in", tag="rh_in")
        nc.sync.dma_start(out=rh[:], in_=w_in_v[ko])
        for sl in range(SPC):
            nc.tensor.matmul(psum_zT[sl][:], rh[:, bass.ts(sl, ED)],
                             hnT_full[:, :, ko, :],
                             start=(ko == 0), stop=(ko == KT_IN - 1))

    # ---------------- Stage E: VQ on local slots ----------------
    binT_sb = sb.tile([ED, SPC], f32)
    nc.sync.dma_start(out=binT_sb[:], in_=t["b_in"][:])
    zT = sb.tile([ED, SPC, B], f32)
    for sl in range(SPC):
        nc.vector.tensor_add(out=zT[:, sl, :], in0=psum_zT[sl][:],
                             in1=binT_sb[:, sl, None].to_broadcast([ED, B]))
    cbT_sb = sb.tile([ED, NE], f32)
    nc.sync.dma_start(out=cbT_sb[:], in_=t["cbT"][:])
    cn_sb = sb.tile([B, NE], f32)
    nc.sync.dma_start(out=cn_sb[:], in_=t["cnorm"][:])
    zqT = sb.tile([ED, SPC, B], f32)
    for sl in range(SPC):
        psc = ps.tile([B, NE], f32, name="psc", tag="psc", bufs=1)
        for nb in range(2):
            nc.tensor.matmul(psc[:, bass.ts(nb, 512)], zT[:, sl, :], cbT_sb[:, bass.ts(nb, 512)],
                             start=True, stop=True)
        scores = sb.tile([B, NE], f32, name="scores", tag="scores")
        nc.vector.tensor_sub(out=scores[:], in0=psc[:], in1=cn_sb[:])
        mx8 = sb.tile([B, 8], f32, name="mx8", tag="mx8")
        nc.vector.max(out=mx8[:], in_=scores[:])
        idx8 = sb.tile([B, 8], u32, name="idx8", tag="idx8")
        nc.vector.max_index(idx8[:], mx8[:], scores[:])
        zq_sl = sb.tile([B, ED], f32, name="zq_sl", tag="zq_sl")
        nc.gpsimd.indirect_dma_start(
            out=zq_sl[:], out_offset=None, in_=t["cb"][:],
            in_offset=bass.IndirectOffsetOnAxis(ap=idx8[:, :1], axis=0))
        pt2 = ps.tile([ED, B], f32, name="pt_zq", tag="pgen")
        nc.tensor.transpose(pt2[:], zq_sl[:], ident[:B, :B])
        nc.vector.tensor_copy(out=zqT[:, sl, :], in_=pt2[:])
    # loss partial: sum((zq - z)^2) * (1+beta)/numel  -> AllReduce
    dif = sb.tile([ED, SPC, B], f32)
    nc.vector.tensor_sub(out=dif[:], in0=zqT[:], in1=zT[:])
    nc.vector.tensor_mul(out=dif[:], in0=dif[:], in1=dif[:])
    dred = sb.tile([ED, 1], f32)
    nc.vector.reduce_sum(dred[:], dif.rearrange("e s b -> e (s b)")[:], axis=AX.X)
    ones128 = sb.tile([128, 1], f32)
    nc.vector.memset(ones128[:], 1.0)
    pl = ps.tile([1, 1], f32, name="pl", tag="pgen")
    nc.tensor.matmul(pl[:], ones128[:], dred[:], start=True, stop=True)
    lp_sb = sb.tile([1, 1], f32)
    nc.vector.tensor_scalar_mul(lp_sb[:], pl[:], (1.0 + BETA) / (B * FLAT_Z))
    nc.sync.dma_start(out=t["loss_part"][:], in_=lp_sb[:])
    # AG zq (also k-major): shard row r = sl*128 + ed -> global k = c*512 + r
    nc.sync.dma_start(out=t["zq_shard"].rearrange("(ki ko) b -> ki ko b", ki=ED), in_=zqT[:])
    nc.gpsimd.collective_compute(
        "AllGather", mybir.AluOpType.bypass, replica_groups=rg,
        ins=[t["zq_shard"][:]], outs=[t["g_zq"][:]])
    nc.gpsimd.collective_compute(
        "AllReduce", mybir.AluOpType.add, replica_groups=rg,
        ins=[t["loss_part"][:]], outs=[t["loss_red"][:]])
    loss_sb = sb.tile([1, 1], f32)
    nc.sync.dma_start(out=loss_sb[:], in_=t["loss_red"][:])
    nc.sync.dma_start(out=t["loss_out"][:], in_=loss_sb[:])

    # ---------------- Stage F: mlp_out (TP): hout[64, 2048] ----------------
    gzq_sb = sb.tile([128, KT_OUT, B], f32r)
    nc.sync.dma_start(out=gzq_sb[:], in_=t["g_zq"].rearrange("(ko ki) b -> ki ko b", ki=128).bitcast(f32r))
    w_out_v = t["w_outT"].rearrange("(ko ki) n -> ko ki n", ki=128)
    psum_h = [ps_acc.tile([B, 512], f32, name=f"psum_h{i}", tag="acc")
              for i in range(4)]
    for ko in range(KT_OUT):
        for nb in range(4):
            rh = wpool.tile([128, 512], f32r, name="rh_out", tag="rh_out")
            nc.sync.dma_start(out=rh[:], in_=w_out_v[ko, :, bass.ts(nb, 512)])
            nc.tensor.matmul(psum_h[nb][:], gzq_sb[:, ko, :], rh[:],
                             start=(ko == 0), stop=(ko == KT_OUT - 1))
    hout = sb.tile([B, HPC], f32)
    bout_sb = sb.tile([B, HPC], f32)
    nc.sync.dma_start(out=bout_sb[:], in_=t["b_out"][:])
    for nb in range(4):
        nc.vector.tensor_add(out=hout[:, bass.ts(nb, 512)], in0=psum_h[nb][:],
                             in1=bout_sb[:, bass.ts(nb, 512)])
    nc.sync.dma_start(out=t["a2a_in"].rearrange("j bl r -> (j bl) r"), in_=hout[:])
    nc.gpsimd.collective_compute(
        "AllToAll", mybir.AluOpType.bypass, replica_groups=rg,
        ins=[t["a2a_in"][:]], outs=[t["a2a_out"][:]])

    # ---------------- Stage G: conv_out (DP over batch) ----------------
    for j in range(NCORES):
        nc.sync.dma_start(
            out=hcat[bass.ts(j, 8), :, :],
            in_=t["a2a_out"][j].rearrange("bl (ch p) -> ch bl p", ch=8))
    scat = sb.tile([128, BPC, PX], f32)
    nc.scalar.activation(scat[:], hcat[:], AF.Silu)
    cow_sb = sb.tile([128, C_OUT], f32)
    nc.sync.dma_start(out=cow_sb[:], in_=t["conv_out_wT"][:])
    cob_sb = sb.tile([128, 2], f32)
    nc.sync.dma_start(out=cob_sb[:], in_=t["conv_out_b"].rearrange("(mt p) -> p mt", p=128))
    sv = scat.rearrange("c b p -> c (b p)")
    for mt in range(2):
        out_mt = sb.tile([128, BPC, PX], f32, name=f"out_mt{mt}", tag=f"out_mt{mt}")
        ov = out_mt.rearrange("o b p -> o (b p)")
        for nb in range(4):
            po = ps.tile([128, 512], f32, name="po", tag="pgen")
            nc.tensor.matmul(po[:], cow_sb[:, bass.ts(mt, 128)], sv[:, bass.ts(nb, 512)],
                             start=True, stop=True)
            nc.vector.tensor_add(out=ov[:, bass.ts(nb, 512)], in0=po[:],
                                 in1=cob_sb[:, mt:mt + 1].to_broadcast([128, 512]))
        nc.sync.dma_start(
            out=t["out_c"][:, bass.ts(mt, 128), :].rearrange("b o p -> o b p"),
            in_=out_mt[:])


def build_program():
    nc = bacc.Bacc("TRN2", target_bir_lowering=False, debug=False,
                   num_devices=NCORES)
    t = {}
    def inp(name, shape):
        t[name] = nc.dram_tensor(name, shape, f32, kind="ExternalInput").ap()
    inp("x_t", [C_IN, NPIX])
    inp("conv_in_wT", [C_IN, CH])
    inp("conv_in_b", [CH])
    inp("ln_w", [FLAT_PH])
    inp("ln_b", [FLAT_PH])
    t["w_inT"] = nc.dram_tensor("w_inT", [FLAT_PH, ZPC], f32r, kind="ExternalInput").ap()
    inp("b_in", [ED, SPC])
    inp("cbT", [ED, NE])
    inp("cb", [NE, ED])
    inp("cnorm", [B, NE])
    t["w_outT"] = nc.dram_tensor("w_outT", [FLAT_Z, HPC], f32r, kind="ExternalInput").ap()
    inp("b_out", [B, HPC])
    inp("conv_out_wT", [CH, C_OUT])
    inp("conv_out_b", [C_OUT])
    t["out_c"] = nc.dram_tensor("out_c", [BPC, C_OUT, PX], f32, kind="ExternalOutput").ap()
    t["loss_out"] = nc.dram_tensor("loss_out", [1, 1], f32, kind="ExternalOutput").ap()
    # internal / shared DRAM for collectives
    t["hn_shard"] = nc.dram_tensor("hn_shard", [FLAT_PH, BPC], f32).ap()
    t["g_hn"] = nc.dram_tensor("g_hn", [NCORES * FLAT_PH, BPC], f32, addr_space="Shared").ap()
    t["zq_shard"] = nc.dram_tensor("zq_shard", [ZPC, B], f32).ap()
    t["g_zq"] = nc.dram_tensor("g_zq", [FLAT_Z, B], f32, addr_space="Shared").ap()
    t["loss_part"] = nc.dram_tensor("loss_part", [1, 1], f32).ap()
    t["loss_red"] = nc.dram_tensor("loss_red", [1, 1], f32, addr_space="Shared").ap()
    t["a2a_in"] = nc.dram_tensor("a2a_in", [NCORES, BPC, HPC], f32).ap()
    t["a2a_out"] = nc.dram_tensor("a2a_out", [NCORES, BPC, HPC], f32).ap()

    from contextlib import ExitStack
    with tile.TileContext(nc) as tc:
        with ExitStack() as ctx:
            _body(tc, t, ctx)
    nc.compile()
    return nc


def host_prep(inputs):
    """Build the 8 per-core input maps from the full-model inputs."""
    f = np.float32
    def c(a):
        return np.ascontiguousarray(a, dtype=f)
    common = {
        "conv_in_wT": c(inputs["conv_in_w"].T),
        "conv_in_b": c(inputs["conv_in_b"]),
        "ln_w": c(inputs["ln_w"].reshape(FLAT_PH)),
        "ln_b": c(inputs["ln_b"].reshape(FLAT_PH)),
        "cbT": c(inputs["codebook"].T),
        "cb": c(inputs["codebook"]),
        "cnorm": c(np.broadcast_to(
            0.5 * (inputs["codebook"].astype(np.float64) ** 2).sum(1).astype(f)[None, :],
            (B, NE))),
        "conv_out_wT": c(inputs["conv_out_w"].T),
        "conv_out_b": c(inputs["conv_out_b"]),
    }
    w_in = inputs["mlp_in_w"]
    w_out = inputs["mlp_out_w"]
    col_order = np.array([ed * 32 + 4 * cc + sl
                          for cc in range(NCORES) for ed in range(ED) for sl in range(SPC)])
    w_out_perm = w_out[:, col_order]
    maps = []
    for cc in range(NCORES):
        d = dict(common)
        xs = inputs["x"][cc * BPC:(cc + 1) * BPC].reshape(BPC, C_IN, PX)
        d["x_t"] = c(xs.transpose(1, 0, 2).reshape(C_IN, NPIX))
        rows = np.array([ed * 32 + 4 * cc + sl for sl in range(SPC) for ed in range(ED)])
        d["w_inT"] = c(w_in[rows, :].T)
        d["b_in"] = c(inputs["mlp_in_b"][rows].reshape(SPC, ED).T)
        d["w_outT"] = c(w_out_perm[cc * HPC:(cc + 1) * HPC, :].T)
        d["b_out"] = c(np.broadcast_to(inputs["mlp_out_b"][cc * HPC:(cc + 1) * HPC][None, :], (B, HPC)))
        maps.append(d)
    return maps


_STATE = {}


def kernel(**inputs):
    if "nc" not in _STATE:
        _STATE["nc"] = build_program()
    nc = _STATE["nc"]
    maps = host_prep(inputs)
    res = run_bass_kernel_spmd(nc, maps, core_ids=list(range(NCORES)))
    _STATE["last_res"] = res
    out = np.empty((B, C_OUT, R, R), dtype=np.float32)
    for cc in range(NCORES):
        out[cc * BPC:(cc + 1) * BPC] = res.results[cc]["out_c"].reshape(BPC, C_OUT, R, R)
    loss = np.float32(res.results[0]["loss_out"][0, 0])
    return out, loss
